# revision 7
# baseline (speedup 1.0000x reference)
"""DCGNN forward kernel for 8 Trainium2 NeuronCores.

The reference network is linear in x (the adjacency is built only from
coord), and the final output is just [B, 2].  The entire pipeline
  x -> Chebyshev(L) -> cheb_W -> (+cheb_b) -> 1x1 conv affine -> FC
therefore collapses to a single affine map

    out[b, n] = sum_k x_flat[b, k] * G[k, n] + const[n],

with G = [C*F_IN, NCLS] = [31744, 2] precomputed on the host from the
tiny parameter tensors (~0.2 MFLOP in f64).  The device kernel is a pure
memory-bound streaming matmul: each core reads its 32.5 MB batch shard
of x exactly once.

Per-core device pipeline (data-parallel over batch, no collectives):
  - DMA x shard in [128, 7936] chunks (4 MB contiguous rows -> ~line rate)
  - PE transpose 128x128 tiles (fp32r, via identity) -> PSUM
  - DVE copy PSUM -> SBUF (two b-halves packed to [128k, 256b])
  - PE matmul accumulate: acc[2, 256] += G_tile[128, 2].T @ xT[128, 256]
    (fp32r: FP22 multiply, fp32 accumulate)
  - matmuls lag transposes by one k-tile so PE never waits on the copy
"""

import numpy as np

_B, _C, _F_IN, _NCLS = 2048, 62, 512, 2
_THRESH = 0.1
_NCORES = 8
_B_LOC = _B // _NCORES            # 256
_KDIM = _C * _F_IN                # 31744
_P = 128
_KT = _KDIM // _P                 # 248 k-tiles
_CHUNK_KT = 62                    # k-tiles per DMA chunk
_NCHUNK = _KT // _CHUNK_KT        # 4
_CHUNK = _CHUNK_KT * _P           # 7936 elements per chunk


def _precompute_g(coord, adj_w1, adj_b1, adj_w2, adj_b2, cheb_W, cheb_b,
                  conv_w, conv_b, fc_w, fc_b):
    """Fold every parameter into G [KDIM, NCLS] and const [NCLS].

    The adjacency MLP + threshold is done in f32 to mirror the reference
    bit-for-bit (the > 0.1 threshold must see the same values); the
    Laplacian / Chebyshev / folding run in f64 for accuracy.
    """
    f32 = np.float32
    coord = coord.astype(f32)
    h = np.maximum(coord @ adj_w1.astype(f32) + adj_b1.astype(f32), f32(0))
    w_star = (h @ adj_w2.astype(f32) + adj_b2.astype(f32))[..., 0]   # [C, C]

    C = w_star.shape[0]
    wd = w_star.astype(np.float64)
    eye = np.eye(C, dtype=bool)
    A = np.where((wd > _THRESH) & ~eye, wd, 0.0)
    deg = A.sum(axis=1)
    dis = np.where(deg > 0, 1.0 / np.sqrt(np.where(deg > 0, deg, 1.0)), 0.0)
    L = -(dis[:, None] * A * dis[None, :])

    K = cheb_W.shape[0]
    T = np.zeros((K, C, C))
    T[0] = np.eye(C)
    T[1] = L
    for k in range(2, K):
        T[k] = 2.0 * (L @ T[k - 1]) - T[k - 2]

    ncls = fc_w.shape[1]
    Fc = fc_w.astype(np.float64).reshape(C, -1, ncls)               # [C, F_OUT, N]
    cw = float(np.asarray(conv_w).reshape(-1)[0])
    cb = float(np.asarray(conv_b).reshape(-1)[0])

    G = np.zeros((C, cheb_W.shape[1], ncls))
    for k in range(K):
        U = np.einsum('if,cfn->icn', cheb_W[k].astype(np.float64), Fc,
                      optimize=True)
        G += np.einsum('cj,icn->jin', T[k], U, optimize=True)
    G *= cw

    const = ((cw * np.tile(cheb_b.astype(np.float64), C) + cb)
             @ fc_w.astype(np.float64)) + fc_b.astype(np.float64)
    return G.reshape(C * cheb_W.shape[1], ncls).astype(f32), const.astype(f32)


_NC_CACHE = None


def _build_nc():
    global _NC_CACHE
    if _NC_CACHE is not None:
        return _NC_CACHE

    import concourse.mybir as mybir
    import concourse.tile as tile
    from concourse import bacc
    from concourse.masks import make_identity

    f32 = mybir.dt.float32
    f32r = mybir.dt.float32r

    # Bacc (not plain Bass): its finalize() runs the TRN2 sync-wait
    # legalization (split >1-wait instructions, move matmul waits to
    # LDWEIGHTS) that walrus codegen requires.
    nc = bacc.Bacc()
    x_dram = nc.declare_dram_parameter("x_shard", [_B_LOC, _KDIM], f32,
                                       isOutput=False)
    g_dram = nc.declare_dram_parameter("g", [_P, _KT * _NCLS], f32,
                                       isOutput=False)
    out_dram = nc.declare_dram_parameter("out_t", [_NCLS, _B_LOC], f32,
                                         isOutput=True)

    with tile.TileContext(nc) as tc:
        with (
            tc.tile_pool(name="const", bufs=1) as const_pool,
            tc.tile_pool(name="x", bufs=2) as x_pool,
            tc.tile_pool(name="at", bufs=3) as at_pool,
            tc.tile_pool(name="tps", bufs=3, space="PSUM") as tpsum_pool,
            tc.tile_pool(name="acc", bufs=1, space="PSUM") as acc_pool,
        ):
            ident = const_pool.tile([_P, _P], f32, tag="ident")
            make_identity(nc, ident[:])

            g_sb = const_pool.tile([_P, _KT * _NCLS], f32, tag="g")
            nc.sync.dma_start(out=g_sb[:], in_=g_dram[:])
            # fp32r operands must come from a producer that rounds to fp32r;
            # a DVE copy into an f32r tile does exactly that.
            g_r = const_pool.tile([_P, _KT * _NCLS], f32r, tag="gr")
            nc.vector.tensor_copy(g_r[:], g_sb[:])

            acc = acc_pool.tile([_NCLS, _B_LOC], f32)

            prev = None  # (at_tile, kt) lagging by one k-tile
            for c in range(_NCHUNK):
                x0 = x_pool.tile([_P, _CHUNK], f32, tag="x0")
                x1 = x_pool.tile([_P, _CHUNK], f32, tag="x1")
                nc.sync.dma_start(
                    out=x0[:], in_=x_dram[0:_P, c * _CHUNK:(c + 1) * _CHUNK])
                nc.sync.dma_start(
                    out=x1[:], in_=x_dram[_P:2 * _P, c * _CHUNK:(c + 1) * _CHUNK])
                for s in range(_CHUNK_KT):
                    kt = c * _CHUNK_KT + s
                    tp = tpsum_pool.tile([_P, 2 * _P], f32, tag="tp")
                    nc.tensor.transpose(
                        tp[:, 0:_P], x0[:, s * _P:(s + 1) * _P], ident[:])
                    nc.tensor.transpose(
                        tp[:, _P:2 * _P], x1[:, s * _P:(s + 1) * _P], ident[:])
                    at = at_pool.tile([_P, 2 * _P], f32r, tag="at")
                    nc.vector.tensor_copy(at[:], tp[:])
                    if prev is not None:
                        pat, pkt = prev
                        nc.tensor.matmul(
                            acc[:], g_r[:, pkt * _NCLS:(pkt + 1) * _NCLS],
                            pat[:], start=(pkt == 0), stop=False)
                    prev = (at, kt)

            pat, pkt = prev
            nc.tensor.matmul(
                acc[:], g_r[:, pkt * _NCLS:(pkt + 1) * _NCLS], pat[:],
                start=False, stop=True)

            out_sb = const_pool.tile([_NCLS, _B_LOC], f32, tag="out")
            nc.vector.tensor_copy(out_sb[:], acc[:])
            nc.sync.dma_start(out=out_dram[:], in_=out_sb[:])

    # Bacc.finalize runs the legalization pipeline (sync-wait splitting,
    # matmul->LDWEIGHTS wait moves, register allocation).
    nc.finalize()

    _NC_CACHE = nc
    return nc


def kernel(x, coord, adj_w1, adj_b1, adj_w2, adj_b2, cheb_W, cheb_b,
           conv_w, conv_b, fc_w, fc_b):
    from concourse.bass_utils import run_bass_kernel_spmd

    g_flat, const = _precompute_g(coord, adj_w1, adj_b1, adj_w2, adj_b2,
                                  cheb_W, cheb_b, conv_w, conv_b, fc_w, fc_b)
    # Device layout: g_host[p, t*NCLS + n] = G[t*128 + p, n]
    g_host = np.ascontiguousarray(
        g_flat.reshape(_KT, _P, _NCLS).transpose(1, 0, 2).reshape(_P, -1))

    x_flat = np.asarray(x, dtype=np.float32).reshape(_B, _KDIM)
    in_maps = [
        {
            "x_shard": np.ascontiguousarray(
                x_flat[i * _B_LOC:(i + 1) * _B_LOC]),
            "g": g_host,
        }
        for i in range(_NCORES)
    ]

    nc = _build_nc()
    res = run_bass_kernel_spmd(nc, in_maps, core_ids=list(range(_NCORES)))
    global _LAST_RESULTS
    _LAST_RESULTS = res

    out = np.concatenate([r["out_t"].T for r in res.results], axis=0)
    return (out + const[None, :]).astype(np.float32)


_LAST_RESULTS = None


# revision 11
# speedup vs baseline: 6.7037x; 6.7037x over previous
"""DCGNN forward kernel for 8 Trainium2 NeuronCores.

The reference network is linear in x (the adjacency is built only from
coord), and the final output is just [B, 2].  The entire pipeline
  x -> Chebyshev(L) -> cheb_W -> (+cheb_b) -> 1x1 conv affine -> FC
therefore collapses to a single affine map

    out[b, n] = sum_k x_flat[b, k] * G[k, n] + const[n],

with G = [C*F_IN, NCLS] = [31744, 2] precomputed on the host from the
tiny parameter tensors (~0.2 MFLOP in f64).  The device kernel is a pure
memory-bound streaming matmul: each core reads its 32.5 MB batch shard
of x exactly once.

Per-core device pipeline (data-parallel over batch, no collectives):
  - DMA x shard in [128, 7936] chunks (4 MB contiguous rows -> ~line rate)
  - PE transpose 128x128 tiles (fp32r, via identity) -> PSUM
  - DVE copy PSUM -> SBUF (two b-halves packed to [128k, 256b])
  - PE matmul accumulate: acc[2, 256] += G_tile[128, 2].T @ xT[128, 256]
    (fp32r: FP22 multiply, fp32 accumulate)
  - matmuls lag transposes by one k-tile so PE never waits on the copy
"""

import numpy as np

_B, _C, _F_IN, _NCLS = 2048, 62, 512, 2
_THRESH = 0.1
_NCORES = 8
_B_LOC = _B // _NCORES            # 256
_KDIM = _C * _F_IN                # 31744
_P = 128
_KT = _KDIM // _P                 # 248 k-tiles
_CHUNK_KT = 62                    # k-tiles per DMA chunk
_NCHUNK = _KT // _CHUNK_KT        # 4
_CHUNK = _CHUNK_KT * _P           # 7936 elements per chunk


def _precompute_g(coord, adj_w1, adj_b1, adj_w2, adj_b2, cheb_W, cheb_b,
                  conv_w, conv_b, fc_w, fc_b):
    """Fold every parameter into G [KDIM, NCLS] and const [NCLS].

    The adjacency MLP + threshold is done in f32 to mirror the reference
    bit-for-bit (the > 0.1 threshold must see the same values); the
    Laplacian / Chebyshev / folding run in f64 for accuracy.
    """
    f32 = np.float32
    coord = coord.astype(f32)
    h = np.maximum(coord @ adj_w1.astype(f32) + adj_b1.astype(f32), f32(0))
    w_star = (h @ adj_w2.astype(f32) + adj_b2.astype(f32))[..., 0]   # [C, C]

    C = w_star.shape[0]
    wd = w_star.astype(np.float64)
    eye = np.eye(C, dtype=bool)
    A = np.where((wd > _THRESH) & ~eye, wd, 0.0)
    deg = A.sum(axis=1)
    dis = np.where(deg > 0, 1.0 / np.sqrt(np.where(deg > 0, deg, 1.0)), 0.0)
    L = -(dis[:, None] * A * dis[None, :])

    K = cheb_W.shape[0]
    T = np.zeros((K, C, C))
    T[0] = np.eye(C)
    T[1] = L
    for k in range(2, K):
        T[k] = 2.0 * (L @ T[k - 1]) - T[k - 2]

    ncls = fc_w.shape[1]
    Fc = fc_w.astype(np.float64).reshape(C, -1, ncls)               # [C, F_OUT, N]
    cw = float(np.asarray(conv_w).reshape(-1)[0])
    cb = float(np.asarray(conv_b).reshape(-1)[0])

    G = np.zeros((C, cheb_W.shape[1], ncls))
    for k in range(K):
        U = np.einsum('if,cfn->icn', cheb_W[k].astype(np.float64), Fc,
                      optimize=True)
        G += np.einsum('cj,icn->jin', T[k], U, optimize=True)
    G *= cw

    const = ((cw * np.tile(cheb_b.astype(np.float64), C) + cb)
             @ fc_w.astype(np.float64)) + fc_b.astype(np.float64)
    return G.reshape(C * cheb_W.shape[1], ncls).astype(f32), const.astype(f32)


_NC_CACHE = {}


def _build_nc(reps=1):
    """Build the bass module. reps>1 emits the whole pipeline that many
    times back-to-back (same I/O) — used only for steady-state timing."""
    if reps in _NC_CACHE:
        return _NC_CACHE[reps]

    import concourse.mybir as mybir
    import concourse.tile as tile
    from concourse import bacc
    from concourse.masks import make_identity

    f32 = mybir.dt.float32
    f32r = mybir.dt.float32r

    # Bacc (not plain Bass): its finalize() runs the TRN2 sync-wait
    # legalization (split >1-wait instructions, move matmul waits to
    # LDWEIGHTS) that walrus codegen requires.
    nc = bacc.Bacc()
    x_dram = nc.declare_dram_parameter("x_shard", [_B_LOC, _KDIM], f32,
                                       isOutput=False)
    g_dram = nc.declare_dram_parameter("g", [_P, _KT * _NCLS], f32,
                                       isOutput=False)
    out_dram = nc.declare_dram_parameter("out_t", [_NCLS, _B_LOC], f32,
                                         isOutput=True)

    with tile.TileContext(nc) as tc:
        with (
            tc.tile_pool(name="const", bufs=1) as const_pool,
            tc.tile_pool(name="x", bufs=2) as x_pool,
            tc.tile_pool(name="at", bufs=3) as at_pool,
            tc.tile_pool(name="tps", bufs=3, space="PSUM") as tpsum_pool,
            tc.tile_pool(name="acc", bufs=1, space="PSUM") as acc_pool,
        ):
            ident = const_pool.tile([_P, _P], f32, tag="ident")
            make_identity(nc, ident[:])

            g_sb = const_pool.tile([_P, _KT * _NCLS], f32, tag="g")
            nc.sync.dma_start(out=g_sb[:], in_=g_dram[:])
            # fp32r operands must come from a producer that rounds to fp32r;
            # a DVE copy into an f32r tile does exactly that.
            g_r = const_pool.tile([_P, _KT * _NCLS], f32r, tag="gr")
            nc.vector.tensor_copy(g_r[:], g_sb[:])

            def one_pass():
                acc = acc_pool.tile([_NCLS, _B_LOC], f32)
                prev = None  # (at_tile, kt) lagging by one k-tile
                for c in range(_NCHUNK):
                    x0 = x_pool.tile([_P, _CHUNK], f32, tag="x0")
                    x1 = x_pool.tile([_P, _CHUNK], f32, tag="x1")
                    nc.sync.dma_start(
                        out=x0[:],
                        in_=x_dram[0:_P, c * _CHUNK:(c + 1) * _CHUNK])
                    nc.sync.dma_start(
                        out=x1[:],
                        in_=x_dram[_P:2 * _P, c * _CHUNK:(c + 1) * _CHUNK])
                    for s in range(_CHUNK_KT):
                        kt = c * _CHUNK_KT + s
                        tp = tpsum_pool.tile([_P, 2 * _P], f32, tag="tp")
                        nc.tensor.transpose(
                            tp[:, 0:_P], x0[:, s * _P:(s + 1) * _P], ident[:])
                        nc.tensor.transpose(
                            tp[:, _P:2 * _P], x1[:, s * _P:(s + 1) * _P],
                            ident[:])
                        at = at_pool.tile([_P, 2 * _P], f32r, tag="at")
                        nc.vector.tensor_copy(at[:], tp[:])
                        if prev is not None:
                            pat, pkt = prev
                            nc.tensor.matmul(
                                acc[:], g_r[:, pkt * _NCLS:(pkt + 1) * _NCLS],
                                pat[:], start=(pkt == 0), stop=False)
                        prev = (at, kt)

                pat, pkt = prev
                nc.tensor.matmul(
                    acc[:], g_r[:, pkt * _NCLS:(pkt + 1) * _NCLS], pat[:],
                    start=False, stop=True)

                out_sb = const_pool.tile([_NCLS, _B_LOC], f32, tag="out")
                nc.vector.tensor_copy(out_sb[:], acc[:])
                nc.sync.dma_start(out=out_dram[:], in_=out_sb[:])

            for _rep in range(reps):
                one_pass()

    # Bacc.finalize runs the legalization pipeline (sync-wait splitting,
    # matmul->LDWEIGHTS wait moves, register allocation).
    nc.finalize()

    _NC_CACHE[reps] = nc
    return nc


def kernel(x, coord, adj_w1, adj_b1, adj_w2, adj_b2, cheb_W, cheb_b,
           conv_w, conv_b, fc_w, fc_b):
    from concourse.bass_utils import run_bass_kernel_spmd

    g_flat, const = _precompute_g(coord, adj_w1, adj_b1, adj_w2, adj_b2,
                                  cheb_W, cheb_b, conv_w, conv_b, fc_w, fc_b)
    # Device layout: g_host[p, t*NCLS + n] = G[t*128 + p, n]
    g_host = np.ascontiguousarray(
        g_flat.reshape(_KT, _P, _NCLS).transpose(1, 0, 2).reshape(_P, -1))

    x_flat = np.asarray(x, dtype=np.float32).reshape(_B, _KDIM)
    in_maps = [
        {
            "x_shard": np.ascontiguousarray(
                x_flat[i * _B_LOC:(i + 1) * _B_LOC]),
            "g": g_host,
        }
        for i in range(_NCORES)
    ]

    nc = _build_nc()
    res = run_bass_kernel_spmd(nc, in_maps, core_ids=list(range(_NCORES)))
    global _LAST_RESULTS
    _LAST_RESULTS = res

    out = np.concatenate([r["out_t"].T for r in res.results], axis=0)
    return (out + const[None, :]).astype(np.float32)


_LAST_RESULTS = None
